# revision 7
# baseline (speedup 1.0000x reference)
"""MIL gated-attention pooling kernel for Trainium2 (8 NeuronCores, SPMD).

Problem (per reference):
    A_pre = tanh(x@W1 + b1) * sigmoid(x@W3 + b3)      # [N, H]
    A     = A_pre @ W2 + b2                           # [N, K]
    P     = softmax over instances per (bag, head)    # [B, K, L]
    out   = einsum('bkl,bld->bkd', P, x) -> [B, K*D]

Shapes hardcoded: B=32 bags, L=2048 instances/bag, D=512, H=256, K=4.
Sharding: data-parallel over bags, 4 bags (8192 rows) per core, weights
replicated. No cross-core communication.

Device algorithm per core (v2 — superstep-fused):
  - 8 supersteps of 1024 instances; the gated-MLP hidden tiles live in
    2-bank PSUM tiles [128, 2, 512] so each tanh covers 1024 columns in
    ONE activation instruction (the per-instruction ACT overhead is
    ~185ns; halving the instruction count buys ~6us of ACT time).
  - sigmoid(h) = 0.5*(1+tanh(h/2)); 0.5 folded into W2 host-side, and
    the gate (1+s)*t is ONE DVE scalar_tensor_tensor op.
  - b2 and softmax max-subtraction dropped (both cancel in softmax).
  - exp() runs once per bag over the whole [128, 16*K] logit block.
  - A@W2 and the weighted-sum matmuls are interleaved between the GEMM's
    DoubleRow matmuls so their LDWEIGHTS hide under big matmuls and the
    PE stays busy (p-state ramp: idle gaps drop the PE clock 2.4->1.2GHz).
  - MODE "bf16": x for the weighted sum streams as bf16 [12.6MB/core].
    MODE "fp8": it streams as fp8 [8.4MB/core] and exp-weights are split
    e = e1 + e2 (both fp8) so the DoubleRow weighted sum loses only x's
    fp8 quantization, not e's.
"""

import numpy as np
import ml_dtypes
from contextlib import ExitStack

B, L, D, H, K = 32, 2048, 512, 256, 4
NCORES = 8
BPC = B // NCORES       # bags per core = 4
R = BPC * L             # rows per core = 8192
SS = 8                  # supersteps per core (1024 instances each)
NI = 1024               # instances per superstep
DC = D // 128           # contraction chunks = 4
NCH = L // 128          # 128-row chunks per bag = 16

_BF16 = ml_dtypes.bfloat16
_FP8 = ml_dtypes.float8_e4m3

MODE = "fp8"            # "bf16" (safe) | "fp8" (fast; err ~1.7e-2 vs 2e-2 gate)
_CACHE = {}


def _build_nc(mode):
    import concourse.bacc as bacc
    import concourse.tile as tile
    import concourse.mybir as mybir
    import concourse.bass as bass

    dt = mybir.dt
    AF = mybir.ActivationFunctionType
    DR = mybir.MatmulPerfMode.DoubleRow
    fp8 = mode == "fp8"

    nc = bacc.Bacc("TRN2", target_bir_lowering=False, debug=False)

    # xt[ss, p, 2*dc2+r, n] = fp8(x[ss*1024+n, dc2*256+r*128+p])
    xt_d = nc.dram_tensor("xt", [SS, 128, DC, NI], dt.float8e4, kind="ExternalInput").ap()
    if fp8:
        # q1[ss, p, pr, j, d] = fp8(x[ss*1024+(2*pr+j)*128+p, d])
        xa_d = nc.dram_tensor("xa", [SS, 128, 4, 2, D], dt.float8e4, kind="ExternalInput").ap()
        ones_d = nc.dram_tensor("ones", [128, 2, 1], dt.float8e4, kind="ExternalInput").ap()
    else:
        # xa[ss, p, c, d] = bf16(x[ss*1024+c*128+p, d])
        xa_d = nc.dram_tensor("xa", [SS, 128, 8, D], dt.bfloat16, kind="ExternalInput").ap()
        ones_d = nc.dram_tensor("ones", [128, 1], dt.bfloat16, kind="ExternalInput").ap()
    # w13[p, dc2, r, h'] = 16*[W1|W3][dc2*256+r*128+p, h']
    w13_d = nc.dram_tensor("w13", [128, 2, 2, 2 * H], dt.float8e4, kind="ExternalInput").ap()
    w2_d = nc.dram_tensor("w2", [128, 2, K], dt.bfloat16, kind="ExternalInput").ap()
    b13_d = nc.dram_tensor("b13", [128, DC], dt.float32, kind="ExternalInput").ap()
    out_d = nc.dram_tensor("out", [BPC, K, D], dt.float32, kind="ExternalOutput").ap()

    with tile.TileContext(nc) as tc, ExitStack() as ctx:
        consts = ctx.enter_context(tc.tile_pool(name="consts", bufs=1))
        xtp = ctx.enter_context(tc.tile_pool(name="xtp", bufs=3))
        xap = ctx.enter_context(tc.tile_pool(name="xap", bufs=6))
        tsp = ctx.enter_context(tc.tile_pool(name="tsp", bufs=4))
        app = ctx.enter_context(tc.tile_pool(name="app", bufs=4))
        epool = ctx.enter_context(tc.tile_pool(name="epool", bufs=2))
        opool = ctx.enter_context(tc.tile_pool(name="opool", bufs=2))
        rpool = ctx.enter_context(tc.tile_pool(name="rpool", bufs=2))

        psH = ctx.enter_context(tc.tile_pool(name="psH", bufs=2, space=bass.MemorySpace.PSUM))
        psA = ctx.enter_context(tc.tile_pool(name="psA", bufs=2, space=bass.MemorySpace.PSUM))
        psU = ctx.enter_context(tc.tile_pool(name="psU", bufs=1, space=bass.MemorySpace.PSUM))
        psZ = ctx.enter_context(tc.tile_pool(name="psZ", bufs=1, space=bass.MemorySpace.PSUM))

        # constants (first w13 half early so the first GEMM can start ASAP)
        w13_sb = consts.tile([128, 2, 2, 2 * H], dt.float8e4)
        nc.sync.dma_start(out=w13_sb[:, 0], in_=w13_d[:, 0])
        b13_sb = consts.tile([128, DC], dt.float32)
        nc.sync.dma_start(out=b13_sb[:], in_=b13_d[:])
        nc.sync.dma_start(out=w13_sb[:, 1], in_=w13_d[:, 1])
        w2_sb = consts.tile([128, 2, K], dt.bfloat16)
        nc.sync.dma_start(out=w2_sb[:], in_=w2_d[:])
        if fp8:
            ones_sb = consts.tile([128, 2, 1], dt.float8e4)
        else:
            ones_sb = consts.tile([128, 1], dt.bfloat16)
        nc.sync.dma_start(out=ones_sb[:], in_=ones_d[:])

        # per-superstep input tiles, prefetched two supersteps ahead
        xt_tiles = {}
        xa_tiles = {}

        def fetch(ss):
            if ss >= SS or ss in xt_tiles:
                return
            xtt = xtp.tile([128, DC, NI], dt.float8e4, tag="xt")
            nc.sync.dma_start(out=xtt[:], in_=xt_d[ss])
            xt_tiles[ss] = xtt
            if fp8:
                xat = xap.tile([128, 4, 2, D], dt.float8e4, tag="xa")
            else:
                xat = xap.tile([128, 8, D], dt.bfloat16, tag="xa")
            nc.sync.dma_start(out=xat[:], in_=xa_d[ss])
            xa_tiles[ss] = xat

        # state carried across supersteps
        apts = {}       # ss -> (apt0, apt1)  [128, 2, 512] bf16, h-chunk pair
        a_ps_of = {}    # bag -> psum [128, NCH*K]
        e_of = {}       # bag -> exp tiles (bf16) or (e1, e2) fp8
        uz_of = {}      # bag -> (u_ps, z_ps)

        def a_w2_quarter(ss, k):
            """A@W2 for chunks 2k, 2k+1 of superstep ss's apt tiles."""
            bag = ss // 2
            if bag not in a_ps_of:
                a_ps_of[bag] = psA.tile([128, NCH * K], dt.float32, tag="a", name="a_ps")
            a_ps = a_ps_of[bag]
            apt0, apt1 = apts[ss]
            for c in (2 * k, 2 * k + 1):
                half, cc = c // 4, c % 4
                slot = (ss % 2) * 8 + c
                nc.tensor.matmul(
                    a_ps[:, K * slot:K * (slot + 1)],
                    apt0[:, half, cc * 128:(cc + 1) * 128],
                    w2_sb[:, 0, :], start=True, stop=False,
                )
                nc.tensor.matmul(
                    a_ps[:, K * slot:K * (slot + 1)],
                    apt1[:, half, cc * 128:(cc + 1) * 128],
                    w2_sb[:, 1, :], start=False, stop=True,
                )

        def do_exp(bag):
            a_ps = a_ps_of.pop(bag)
            e_b = epool.tile([128, NCH, K], dt.bfloat16, tag="e")
            nc.scalar.activation(e_b[:], a_ps[:], AF.Exp)
            if fp8:
                # walrus rejects DoubleRow lhsT narrower than 16 cols, so e1/e2
                # are zero-padded [*, NCH, 16]; rows 4..15 of u/z stay zero.
                e1 = epool.tile([128, NCH, 16], dt.float8e4, tag="e1")
                nc.gpsimd.memset(e1[:, :, K:], 0.0)
                nc.vector.tensor_scalar_add(out=e1[:, :, 0:K], in0=e_b[:], scalar1=0.0)
                e2 = epool.tile([128, NCH, 16], dt.float8e4, tag="e2")
                nc.gpsimd.memset(e2[:, :, K:], 0.0)
                nc.vector.scalar_tensor_tensor(
                    out=e2[:, :, 0:K], in0=e1[:, :, 0:K], scalar=-1.0, in1=e_b[:],
                    op0=mybir.AluOpType.mult, op1=mybir.AluOpType.add,
                )
                e_of[bag] = (e1, e2)
                u_ps = psU.tile([16, D], dt.float32, tag="u")
                z_ps = psZ.tile([16, 1], dt.float32, tag="z")
            else:
                e_of[bag] = e_b
                u_ps = psU.tile([K, D], dt.float32, tag="u")
                z_ps = psZ.tile([K, 1], dt.float32, tag="z")
            uz_of[bag] = (u_ps, z_ps)

        def wsum_quarter(bag, k):
            """Quarter k of bag's exp-weighted sum + normalizer."""
            u_ps, z_ps = uz_of[bag]
            if fp8:
                e1, e2 = e_of[bag]
                for pr in (2 * k, 2 * k + 1):  # chunk-pairs 0..7
                    xat = xa_tiles[2 * bag + pr // 4]
                    pp = pr % 4
                    first, last = pr == 0, pr == 7
                    nc.tensor.matmul(u_ps[:], e1[:, 2 * pr:2 * pr + 2, :], xat[:, pp],
                                     start=first, stop=False, perf_mode=DR)
                    nc.tensor.matmul(u_ps[:], e2[:, 2 * pr:2 * pr + 2, :], xat[:, pp],
                                     start=False, stop=last, perf_mode=DR)
                    nc.tensor.matmul(z_ps[:], e1[:, 2 * pr:2 * pr + 2, :], ones_sb[:],
                                     start=first, stop=False, perf_mode=DR)
                    nc.tensor.matmul(z_ps[:], e2[:, 2 * pr:2 * pr + 2, :], ones_sb[:],
                                     start=False, stop=last, perf_mode=DR)
            else:
                e_b = e_of[bag]
                for c in (4 * k, 4 * k + 1, 4 * k + 2, 4 * k + 3):  # chunks 0..15
                    xat = xa_tiles[2 * bag + c // 8]
                    cc = c % 8
                    first, last = c == 0, c == NCH - 1
                    nc.tensor.matmul(u_ps[:], e_b[:, c, :], xat[:, cc],
                                     start=first, stop=last)
                    nc.tensor.matmul(z_ps[:], e_b[:, c, :], ones_sb[:],
                                     start=first, stop=last)

        def finish(bag):
            del e_of[bag]
            u_ps, z_ps = uz_of.pop(bag)
            r_sb = rpool.tile([K, 1], dt.float32, tag="r")
            nc.vector.reciprocal(out=r_sb[:], in_=z_ps[0:K, :])
            o_sb = opool.tile([K, D], dt.float32, tag="o")
            nc.vector.tensor_scalar_mul(out=o_sb[:], in0=u_ps[0:K, :], scalar1=r_sb[:])
            nc.sync.dma_start(out=out_d[bag], in_=o_sb[:])
            for s in (2 * bag, 2 * bag + 1):
                del xa_tiles[s]

        fetch(0)
        fetch(1)

        for ss in range(SS):
            fetch(ss + 2)
            xtt = xt_tiles.pop(ss)
            tanh_of = {}
            for k, bp in enumerate((0, 2, 1, 3)):
                h2 = psH.tile([128, 2, 512], dt.float32, tag="h")
                for half in range(2):
                    for dc2 in range(2):
                        nc.tensor.matmul(
                            h2[:, half, :],
                            w13_sb[:, dc2, :, bp * 128:(bp + 1) * 128],
                            xtt[:, 2 * dc2:2 * dc2 + 2, half * 512:(half + 1) * 512],
                            start=(dc2 == 0), stop=(dc2 == 1),
                            perf_mode=DR,
                        )
                # deferred PE work rides between the GEMM's matmul groups
                if ss >= 1:
                    a_w2_quarter(ss - 1, k)
                    if ss >= 2 and ss % 2 == 1:
                        wsum_quarter(ss // 2 - 1, k)
                ts = tsp.tile([128, 2, 512], dt.bfloat16, tag="ts")
                nc.scalar.activation(
                    ts[:], h2[:], AF.Tanh, bias=b13_sb[:, bp:bp + 1],
                    scale=(1.0 / 16.0 if bp < 2 else 0.5 / 16.0),
                )
                tanh_of[bp] = ts
                if k == 1:  # t(0) and s(2) ready -> gate h-chunk 0
                    apt0 = app.tile([128, 2, 512], dt.bfloat16, tag="ap")
                    nc.vector.scalar_tensor_tensor(
                        out=apt0[:], in0=tanh_of[2][:], scalar=1.0, in1=tanh_of[0][:],
                        op0=mybir.AluOpType.add, op1=mybir.AluOpType.mult,
                    )
                if k == 3:
                    apt1 = app.tile([128, 2, 512], dt.bfloat16, tag="ap")
                    nc.vector.scalar_tensor_tensor(
                        out=apt1[:], in0=tanh_of[3][:], scalar=1.0, in1=tanh_of[1][:],
                        op0=mybir.AluOpType.add, op1=mybir.AluOpType.mult,
                    )
                    apts[ss] = (apt0, apt1)
            if ss >= 1 and ss % 2 == 0:
                do_exp(ss // 2 - 1)     # logits of bag ss//2-1 complete
                del apts[ss - 1]
            if ss >= 3 and ss % 2 == 1:
                finish(ss // 2 - 1)

        # epilogue: drain the last superstep's A@W2, exp, wsum, finish
        for k in range(4):
            a_w2_quarter(SS - 1, k)
        do_exp(BPC - 1)
        for k in range(4):
            wsum_quarter(BPC - 1, k)
        finish(BPC - 1)

    nc.compile()
    return nc


def get_nc():
    key = "nc_" + MODE
    if key not in _CACHE:
        _CACHE[key] = _build_nc(MODE)
    return _CACHE[key]


def make_in_maps(x, W1, b1, W3, b3, W2, b2):
    x = np.asarray(x, dtype=np.float32)
    W1 = np.asarray(W1, dtype=np.float32)
    W3 = np.asarray(W3, dtype=np.float32)
    W2 = np.asarray(W2, dtype=np.float32)
    b1 = np.asarray(b1, dtype=np.float32)
    b3 = np.asarray(b3, dtype=np.float32)
    fp8 = MODE == "fp8"

    w13 = np.concatenate([W1, W3], axis=1)          # [512, 512]
    # [p, dc2, r, h'] = 16*w13[dc2*256 + r*128 + p, h']
    w13_t = np.ascontiguousarray(
        (16.0 * w13).reshape(2, 2, 128, 2 * H).transpose(2, 0, 1, 3)
    ).astype(_FP8)
    w2_t = np.ascontiguousarray(
        (0.5 * W2).reshape(2, 128, K).transpose(1, 0, 2)
    ).astype(_BF16)
    b13 = np.concatenate([b1, 0.5 * b3]).reshape(DC, 128).T
    b13 = np.ascontiguousarray(b13, dtype=np.float32)
    if fp8:
        ones = np.ones((128, 2, 1), dtype=_FP8)
    else:
        ones = np.ones((128, 1), dtype=_BF16)

    in_maps = []
    for cid in range(NCORES):
        xc = x[cid * R:(cid + 1) * R]               # [8192, 512] fp32
        # xt[ss, p, 2*dc2+r, n] = x[ss*1024+n, dc2*256+r*128+p]
        xt_np = np.ascontiguousarray(
            xc.T.reshape(2, 2, 128, SS, NI).transpose(3, 2, 0, 1, 4).reshape(SS, 128, DC, NI)
        ).astype(_FP8)
        if fp8:
            xa_np = np.ascontiguousarray(
                xc.reshape(SS, 4, 2, 128, D).transpose(0, 3, 1, 2, 4)
            ).astype(_FP8)
        else:
            xa_np = np.ascontiguousarray(
                xc.reshape(SS, 8, 128, D).transpose(0, 2, 1, 3)
            ).astype(_BF16)
        in_maps.append(
            {"xt": xt_np, "xa": xa_np, "w13": w13_t, "w2": w2_t,
             "b13": b13, "ones": ones}
        )
    return in_maps


def kernel(x, W1, b1, W3, b3, W2, b2, bag_lengths):
    from concourse.bass_utils import run_bass_kernel_spmd

    nc = get_nc()
    in_maps = make_in_maps(x, W1, b1, W3, b3, W2, b2)
    res = run_bass_kernel_spmd(nc, in_maps, list(range(NCORES)))
    out = np.empty((B, K * D), dtype=np.float32)
    for c in range(NCORES):
        out[c * BPC:(c + 1) * BPC] = res.results[c]["out"].reshape(BPC, K * D)
    return out
